# revision 34
# baseline (speedup 1.0000x reference)
"""HashLayerFFN expert-parallel Trainium2 kernel (schedule-optimized).

Routing model: each token picks one of E=8 expert FFNs via a hash map.
Host side groups tokens by expert (cheap numpy), pads each expert bucket to
capacity C (= max bucket size for this input set), and gives expert i's
weights + tokens to core i.  All matrices are pre-transposed on the host so
the device kernel is two dense back-to-back matmul phases:

  phase 1:  HidT[h, c] = relu( sum_d W1T[d, h] * XT[d, c] + b1[h] )
  phase 2:  YT[d, c]   =       sum_h W2T[h, d] * HidT[h, c] + b2[d]

Schedule notes (driven by the TimelineSim cost model):
 - PE p-state ramps 0.65->1.2->2.4GHz over 3us of CONTINUOUS execution and
   resets on ANY idle gap, so the kernel opens with a chain of zero-data
   warmup matmuls sized to end exactly when the first real operands' DMA
   semaphores fire; small zero-adding pad matmuls (start=False into the
   open accumulation group) paper over sub-100ns relu-handoff stalls at
   wave boundaries.
 - The weight stream (8.4MB fp16 per core, used once) starts on two queues
   in parallel (DVE carries the first W1 tile, SP everything else) so the
   first matmul can fire ~3.2us in (DGE pipeline latency + 900ns DMA-sem
   propagation are unavoidable).
 - Phase 1 runs as two 8-high d-major waves (consumption 1033ns per
   256KB+80KB chunk vs 948ns supply at 360GB/s, so PE stays ahead); each
   wave ends with a 2-column h-major tail so the 8 PSUM banks close
   staggered and the relus (alternating Act/DVE) can hand banks to the
   next wave without stalling PE.
 - Phase 2 sweeps h-major while W2 streams, then switches group-major so
   the 8 output banks close 1us apart and the store chains (act bias+cast
   to fp16 -> HWDGE 625 -> DGE 650 -> DMA -> 900ns sem) overlap all but
   the last ~3.8us.
"""

import numpy as np

B, S, D, H, E = 2, 1024, 1024, 2048, 8
N_CORES = 8
DT = 16            # h tiles of 128 in H
ND = 8             # d chunks of 128 in D

# matmul dtype mode "layer1_layer2" (fp16 measured 4.1e-04 rel err)
MODE = "fp16_fp16"
# W1 as float8 e3m4 scaled by W1_SCALE (b1 scaled up, W2 scaled down on the
# host so the kernel math is unchanged); halves the wave-1 weight stream.
W1_FP8 = True
W1_SCALE = 256.0

# warmup tuning: long dummies (C rows) then short dummies (SHORT rows)
WARM_SHORT = 103
SHORT = 32
# zero-adding pad matmuls (rows) inserted at wave handoffs: list of
# (wave, col, pos, rows) consumed by the builder; tuned against sim.
PADS_WAVE = {}  # {wave: [(col, h_pos, n_pads)]}
PADS_TAIL = {}      # {wave: n_pads} pads before the h-major tail
PADS_P2C0 = []    # [(sweep_pos, n_pads)] pads in first phase-2 sweep

RUN_KWARGS = {}
LAST_RES = None

_cache = {}
_last_nc = None


def _np_dt(name):
    if name == "bf16":
        import ml_dtypes
        return np.dtype(ml_dtypes.bfloat16)
    if name == "fp16":
        return np.dtype(np.float16)
    return np.dtype(np.float32)


def _build_nc(mode, C):
    import concourse.mybir as mybir
    from concourse import bacc
    from concourse.tile import TileContext

    f32 = mybir.dt.float32
    mmdt = {
        "f32r": mybir.dt.float32r,
        "f32": f32,
        "fp16": mybir.dt.float16,
        "bf16": mybir.dt.bfloat16,
    }
    l1, l2 = mode.split("_")
    dt1, dt2 = mmdt[l1], mmdt[l2]
    dtw1 = mybir.dt.float8e3 if W1_FP8 else dt1

    nc = bacc.Bacc(None, target_bir_lowering=False)
    xt = nc.dram_tensor("xt", [128, ND, C], dt1, kind="ExternalInput")
    w1t = nc.dram_tensor("w1t", [ND, 128, 8, 128], dtw1, kind="ExternalInput")
    w1t2 = nc.dram_tensor("w1t2", [ND, 128, 8, 128], dt1, kind="ExternalInput")
    bt = nc.dram_tensor("bt", [128, DT + ND], f32, kind="ExternalInput")
    w2t = nc.dram_tensor("w2t", [DT, 128, D], dt2, kind="ExternalInput")
    yt = nc.dram_tensor("yt", [ND, 128, C], dt2, kind="ExternalOutput")

    with TileContext(nc) as tc:
        with (
            tc.tile_pool(name="consts", bufs=1) as consts,
            tc.tile_pool(name="dpool", bufs=1) as dpool,
            tc.tile_pool(name="xpool", bufs=1) as xpool,
            tc.tile_pool(name="w1pool", bufs=1) as w1pool,
            tc.tile_pool(name="w2pool", bufs=1) as w2pool,
            tc.tile_pool(name="hpool", bufs=1) as hpool,
            tc.tile_pool(name="ypool", bufs=4) as ypool,
            tc.tile_pool(name="psP", bufs=1, space="PSUM") as psP,
        ):
            # ---- warmup: PE busy from ~70ns so the clock is fully ramped
            # (3us of continuous execution) when real operands arrive.
            dummy = dpool.tile([128, 128 + SHORT], dt1, name="dummy")
            nc.vector.memset(dummy, 0.0)
            psts = [psP.tile([128, C], f32, name=f"ps{j}") for j in range(8)]
            psD = psts[0]
            for i in range(WARM_SHORT):
                nc.tensor.matmul(psD[:, 0:SHORT], lhsT=dummy[:, 0:128],
                                 rhs=dummy[:, 128:128 + SHORT], start=True, stop=True)

            def pad(group_tile, rows):
                # zero-adding filler: keeps PE busy across a sub-100ns
                # dependency stall without resetting the p-state ramp
                nc.tensor.matmul(group_tile[:, 0:rows], lhsT=dummy[:, 0:128],
                                 rhs=dummy[:, 128:128 + rows], start=False, stop=False)

            # ---- weight/x stream.  Wave 1 is supply-bound end-to-end, so
            # it uses few, large DMAs: xt as ONE gpsimd-queue DMA (SWDGE
            # desc-gen runs on the idle Pool engine, parallel to HWDGE),
            # W1 g0 as 8 per-d tiles, W1 g1 / W2 as 2-wide tiles.
            w1g0 = []
            w1g0.append(w1pool.tile([128, 8, 128], dtw1, name="w1_0"))
            nc.sync.dma_start(out=w1g0[0], in_=w1t[0, :, :, :])
            xtile = xpool.tile([128, ND, C], dt1, name="xtile")
            xts = [xtile[:, d, :] for d in range(ND)]
            nc.sync.dma_start(out=xtile[:, 0, :], in_=xt[:, 0, :])
            nc.gpsimd.dma_start(out=xtile[:, 1, :], in_=xt[:, 1, :])
            nc.gpsimd.dma_start(out=xtile[:, 2:4, :], in_=xt[:, 2:4, :])
            nc.gpsimd.dma_start(out=xtile[:, 4:8, :], in_=xt[:, 4:8, :])
            for d in range(1, 7):
                w1g0.append(w1pool.tile([128, 8, 128], dtw1, name=f"w1_{d}"))
                nc.sync.dma_start(out=w1g0[d], in_=w1t[d, :, :, :])
            w1d7 = w1pool.tile([128, 8, 128], dtw1, name="w1_7")
            nc.sync.dma_start(out=w1d7[:, 0:4, :], in_=w1t[7, :, 0:4, :])
            nc.sync.dma_start(out=w1d7[:, 4:8, :], in_=w1t[7, :, 4:8, :])
            w1g0.append(w1d7)
            bts = consts.tile([128, DT + ND], f32)
            nc.sync.dma_start(out=bts, in_=bt[:])
            b1s, b2s = bts[:, 0:DT], bts[:, DT:DT + ND]
            w1g1 = [None] * ND
            for d in range(0, ND, 2):
                g1 = w1pool.tile([128, 2, 8, 128], dt1, name=f"w1g1_{d}")
                nc.sync.dma_start(out=g1,
                                  in_=w1t2[d:d + 2, :, :, :].transpose([1, 0, 2, 3]))
                w1g1[d] = g1[:, 0]
                w1g1[d + 1] = g1[:, 1]
            w2s = []
            for h in range(0, DT, 2):
                w2tile = w2pool.tile([128, 2, D], dt2, name=f"w2_{h}")
                nc.sync.dma_start(out=w2tile, in_=w2t[h:h + 2].transpose([1, 0, 2]))
                w2s.append(w2tile[:, 0])
                w2s.append(w2tile[:, 1])

            # ---- phase 1: two 8-high d-major waves; each ends with a
            # 2-column h-major tail so the 8 banks close staggered and the
            # relus (alternating Act/DVE) hand banks over without stalls.
            hids = [None] * DT
            for wi in range(2):
                chunks = w1g0 if wi == 0 else w1g1
                hbase = 8 * wi
                pws = psts
                for d in range(6):
                    for j in range(8):
                        for pc, pj, n in PADS_WAVE.get(wi, []):
                            if pc == d and pj == j:
                                for _ in range(n):
                                    pad(pws[0], SHORT)
                        nc.tensor.matmul(pws[j], lhsT=chunks[d][:, j, :],
                                         rhs=xts[d], start=(d == 0), stop=False)
                for _ in range(PADS_TAIL.get(wi, 0)):
                    pad(pws[0], SHORT)
                for j in range(8):
                    for d in (6, 7):
                        nc.tensor.matmul(pws[j], lhsT=chunks[d][:, j, :],
                                         rhs=xts[d], start=False, stop=(d == 7))
                    relu(nc, mybir, hbase + j, pws[j], hids, hpool, b1s, dt2, C)

            # ---- phase 2: 8 output banks, first h-sweep rotated (d4..d7
            # banks free first), then h-major h1..7, then group-major h8..15
            # per d so banks close 1us apart and stores overlap matmuls.
            pgs = psts
            sweep0 = list(range(8))
            for i, d in enumerate(sweep0):
                for pj, n in PADS_P2C0:
                    if pj == i:
                        for _ in range(n):
                            pad(pgs[sweep0[0]], SHORT)
                nc.tensor.matmul(pgs[d], lhsT=w2s[0][:, d * 128:(d + 1) * 128],
                                 rhs=hids[0], start=True, stop=False)
            for h in range(1, 8):
                for d in range(ND):
                    nc.tensor.matmul(pgs[d], lhsT=w2s[h][:, d * 128:(d + 1) * 128],
                                     rhs=hids[h], start=False, stop=False)
            for d in range(ND):
                for h in range(8, DT):
                    nc.tensor.matmul(pgs[d], lhsT=w2s[h][:, d * 128:(d + 1) * 128],
                                     rhs=hids[h], start=False, stop=(h == DT - 1))
                ysb = ypool.tile([128, C], dt2, name="ysb")
                if d % 2 == 0:
                    nc.scalar.activation(
                        out=ysb, in_=pgs[d],
                        func=mybir.ActivationFunctionType.Identity,
                        bias=b2s[:, d:d + 1],
                    )
                else:
                    nc.vector.tensor_scalar_add(ysb, pgs[d], b2s[:, d:d + 1])
                nc.sync.dma_start(out=yt[d], in_=ysb)

    nc.finalize()
    return nc


def relu(nc, mybir, h, psum_tile, hids, hpool, b1s, dt2, C):
    hid = hpool.tile([128, C], dt2, name=f"hid{h}")
    if h % 2 == 0:
        nc.scalar.activation(
            out=hid, in_=psum_tile,
            func=mybir.ActivationFunctionType.Relu,
            bias=b1s[:, h:h + 1],
        )
    else:
        nc.vector.tensor_scalar(
            out=hid, in0=psum_tile,
            scalar1=b1s[:, h:h + 1], scalar2=0.0,
            op0=mybir.AluOpType.add, op1=mybir.AluOpType.max,
        )
    hids[h] = hid


def _get_nc(mode=None, C=None):
    global _last_nc
    if mode is None:
        return _last_nc
    key = (mode, C, W1_FP8)
    if key not in _cache:
        _cache[key] = _build_nc(mode, C)
    _last_nc = _cache[key]
    return _cache[key]


def kernel(x, orig_input, hash_map, W1, b1, W2, b2, **_unused):
    from concourse import bass_utils

    x = np.asarray(x)
    W1 = np.asarray(W1, dtype=np.float32)
    b1 = np.asarray(b1, dtype=np.float32)
    W2 = np.asarray(W2, dtype=np.float32)
    b2 = np.asarray(b2, dtype=np.float32)
    l1, l2 = MODE.split("_")
    dt1, dt2 = _np_dt(l1), _np_dt(l2)

    xf = np.ascontiguousarray(x, dtype=np.float32).reshape(B * S, D)
    e = np.asarray(hash_map).astype(np.int64)[
        np.asarray(orig_input).astype(np.int64).reshape(-1)
    ]
    order = np.argsort(e, kind="stable")
    counts = np.bincount(e, minlength=E)
    starts = np.zeros(E + 1, dtype=np.int64)
    starts[1:] = np.cumsum(counts)

    C = max(64, int(counts.max() + 1) // 2 * 2)   # even capacity >= max bucket

    in_maps = []
    idxs = []
    for i in range(E):
        idx = order[starts[i]:starts[i + 1]]
        idxs.append(idx)
        xe = np.zeros((C, D), dtype=np.float32)
        xe[: len(idx)] = xf[idx]
        if W1_FP8:
            import ml_dtypes
            w1full = np.ascontiguousarray(W1[i].T * W1_SCALE)\
                       .reshape(ND, 128, DT, 128)
            w1pack = w1full[:, :, 0:8].astype(ml_dtypes.float8_e3m4)
            w1pack2 = np.ascontiguousarray(w1full[:, :, 8:16]).astype(dt1)
            w2pack = np.ascontiguousarray(W2[i].T / W1_SCALE).astype(dt2)\
                       .reshape(DT, 128, D)
            b1pack = (b1[i] * W1_SCALE).reshape(DT, 128).T
        else:
            w1full = np.ascontiguousarray(W1[i].T).astype(dt1)\
                       .reshape(ND, 128, DT, 128)
            w1pack, w1pack2 = w1full[:, :, 0:8].copy(), w1full[:, :, 8:16].copy()
            w2pack = np.ascontiguousarray(W2[i].T).astype(dt2)\
                       .reshape(DT, 128, D)
            b1pack = b1[i].reshape(DT, 128).T
        in_maps.append({
            "xt": np.ascontiguousarray(
                xe.T.reshape(ND, 128, C).transpose(1, 0, 2)).astype(dt1),
            "w1t": w1pack,
            "w1t2": w1pack2,
            "w2t": w2pack,
            "bt": np.ascontiguousarray(np.concatenate(
                [b1pack, b2[i].reshape(ND, 128).T], axis=1)),
        })

    nc = _get_nc(MODE, C)
    res = bass_utils.run_bass_kernel_spmd(
        nc, in_maps, core_ids=list(range(N_CORES)), **RUN_KWARGS
    )
    global LAST_RES
    LAST_RES = res

    out = np.zeros((B * S, D), dtype=np.float32)
    for i in range(E):
        idx = idxs[i]
        y = res.results[i]["yt"].reshape(D, C).T  # [C, D]
        out[idx] = y[: len(idx)].astype(np.float32)
    return out.reshape(B, S, D)
